# revision 18
# baseline (speedup 1.0000x reference)
"""Trainium2 Bass kernel for nn_ActorTanh (gnn_message_passing).

Data-parallel over the batch: 8 NeuronCores x 128 samples each.
Per-sample knn(K=10) over NB=30 nodes + EdgeConv + actor head.

Key algebraic restructure: EdgeConv message
    e_ij = [x_i, x_j - x_i] @ W_m1 = x_i @ (A - B) + x_j @ B   (A,B halves of W_m1)
so per-node u = feat@(A-B)+b_m1 and v = feat@B are computed once, and the
per-edge pre-activation is z_ij = u_i + v_j (gathered), cutting the big
[307200,384] matmul to two [30720,192] ones.

Layouts: activations are feature-major [feat<=128 partitions, nodes free].
The edge gather uses gpsimd dma_gather(transpose=True) from a bf16 DRAM
v-table, which lands feature-major directly; the u_i term is added by the
TensorEngine via a one-hot "repeat" matmul accumulating into the same PSUM.
"""

import os
import numpy as np

import concourse.bass as bass
import concourse.mybir as mybir
from concourse import bacc
from concourse.tile import TileContext

BS, NB, K, HID, EMB = 1024, 30, 10, 128, 64
NCORES = 8
BSL = BS // NCORES          # 128 samples per core
NL = BSL * NB               # 3840 nodes per core
NE = NL * K                 # 38400 edges per core
GC = 64                     # centers per edge-group
NG = NL // GC               # 60 edge groups
NT = 480                    # node-tile width for dense MLP matmuls
F32 = mybir.dt.float32
BF16 = mybir.dt.bfloat16
I16 = mybir.dt.int16
U16 = mybir.dt.uint16
AF = mybir.ActivationFunctionType
OP = mybir.AluOpType


def build_nc():
    nc = bacc.Bacc()
    state = nc.declare_dram_parameter("state_inp", [BSL, NB * 2], F32, False)
    tar = nc.declare_dram_parameter("tar_scores", [NL, 2], F32, False)
    W_in1 = nc.declare_dram_parameter("W_in1", [4, HID], F32, False)
    b_in1 = nc.declare_dram_parameter("b_in1", [HID], F32, False)
    W_in2 = nc.declare_dram_parameter("W_in2", [HID, HID], F32, False)
    b_in2 = nc.declare_dram_parameter("b_in2", [HID], F32, False)
    emb_tab = nc.declare_dram_parameter("emb_tab", [3, EMB], F32, False)
    W_emb = nc.declare_dram_parameter("W_emb", [EMB, EMB], F32, False)
    b_emb = nc.declare_dram_parameter("b_emb", [EMB], F32, False)
    W_m1 = nc.declare_dram_parameter("W_m1", [(HID + EMB) * 2, HID], F32, False)
    b_m1 = nc.declare_dram_parameter("b_m1", [HID], F32, False)
    W_m2 = nc.declare_dram_parameter("W_m2", [HID, HID], F32, False)
    b_m2 = nc.declare_dram_parameter("b_m2", [HID], F32, False)
    W_a1 = nc.declare_dram_parameter("W_a1", [HID, HID], F32, False)
    b_a1 = nc.declare_dram_parameter("b_a1", [HID], F32, False)
    W_a2 = nc.declare_dram_parameter("W_a2", [HID, 4], F32, False)
    b_a2 = nc.declare_dram_parameter("b_a2", [4], F32, False)
    out = nc.declare_dram_parameter("out", [2, BSL, NB * 2], F32, True)

    v_dram = nc.dram_tensor("v_tab", [NL, HID], BF16)
    idx_dram = nc.dram_tensor("idx_tab", [NE], I16)

    # allocate the dma_gather count register up front, while the Pool-engine
    # register file is empty (the constant value-cache fills it up later)
    nreg = nc.gpsimd.to_reg(GC * K)

    with TileContext(nc) as tc:
        with tc.tile_pool(name="main", bufs=1) as P:
            # ---------------- constants / weights in SBUF ----------------
            w_in1 = P.tile([4, HID], F32, tag="w_in1")  # rows permuted: (tar, pos)
            nc.sync.dma_start(out=w_in1[0:2, :], in_=W_in1[2:4, :])
            nc.sync.dma_start(out=w_in1[2:4, :], in_=W_in1[0:2, :])
            w_in2 = P.tile([HID, HID], F32, tag="w_in2")
            nc.sync.dma_start(out=w_in2[:], in_=W_in2[:])
            w_emb = P.tile([EMB, EMB], F32, tag="w_emb")
            nc.sync.dma_start(out=w_emb[:], in_=W_emb[:])
            w_a1 = P.tile([HID, HID], F32, tag="w_a1")
            nc.sync.dma_start(out=w_a1[:], in_=W_a1[:])
            w_a2 = P.tile([HID, 4], F32, tag="w_a2")
            nc.sync.dma_start(out=w_a2[:], in_=W_a2[:])

            bi1 = P.tile([HID, 1], F32, tag="bi1")
            nc.sync.dma_start(out=bi1[:], in_=b_in1[:].unsqueeze(1))
            bi2 = P.tile([HID, 1], F32, tag="bi2")
            nc.sync.dma_start(out=bi2[:], in_=b_in2[:].unsqueeze(1))
            bem = P.tile([EMB, 1], F32, tag="bem")
            nc.sync.dma_start(out=bem[:], in_=b_emb[:].unsqueeze(1))
            bm2 = P.tile([HID, 1], F32, tag="bm2")
            nc.sync.dma_start(out=bm2[:], in_=b_m2[:].unsqueeze(1))
            ba1 = P.tile([HID, 1], F32, tag="ba1")
            nc.sync.dma_start(out=ba1[:], in_=b_a1[:].unsqueeze(1))
            ba2 = P.tile([2, 1], F32, tag="ba2")
            nc.sync.dma_start(out=ba2[:], in_=b_a2[0:2].unsqueeze(1))
            ba2ls = P.tile([2, 1], F32, tag="ba2ls")
            nc.sync.dma_start(out=ba2ls[:], in_=b_a2[2:4].unsqueeze(1))

            # W_m1 pieces: A = rows[0:192], B = rows[192:384]
            wm1 = P.tile([128, 384], F32, tag="wm1")  # [part, (4 chunks of 96? no)]
            # load as 3 chunks of 128 rows: rows r -> tile[:, r-chunk]
            w_m1_r = W_m1[:].rearrange("(c p) h -> p c h", p=128)  # c=3
            nc.sync.dma_start(
                out=wm1[:].rearrange("p (c h) -> p c h", c=3), in_=w_m1_r
            )
            # A chunk1 = wm1[:,0:128] rows0-127 ; A chunk2 rows128-191 = wm1[0:64,128:256]
            # B chunk1 rows192-319 = wm1[64:128,128:256] ++ wm1[0:64,256:384]?? no:
            # rows 128..255 -> c=1 (tile cols 128:256, partition = row-128)
            # rows 256..383 -> c=2 (tile cols 256:384)
            # So: A2 (rows 128:192) = wm1[0:64, 128:256]
            #     B1 (rows 192:256) = wm1[64:128, 128:256]
            #     B2 (rows 256:320) = wm1[0:64, 256:384]
            #     B3 (rows 320:384) = wm1[64:128, 256:384]
            # We need B as [128,128]+[64,128] at partition offsets 0/0:
            wb1 = P.tile([128, HID], F32, tag="wb1")  # B rows 0:128 (=W_m1 192:320)
            nc.vector.tensor_copy(out=wb1[0:64, :], in_=wm1[64:128, 128:256])
            nc.vector.tensor_copy(out=wb1[64:128, :], in_=wm1[0:64, 256:384])
            wb2 = P.tile([64, HID], F32, tag="wb2")  # B rows 128:192
            nc.vector.tensor_copy(out=wb2[:], in_=wm1[64:128, 256:384])
            # D = A - B chunks, with b_m1 as extra row on chunk2
            wd1 = P.tile([128, HID], F32, tag="wd1")
            nc.vector.tensor_sub(out=wd1[:], in0=wm1[:, 0:128], in1=wb1[:])
            wd2 = P.tile([65, HID], F32, tag="wd2")
            nc.vector.tensor_sub(out=wd2[0:64, :], in0=wm1[0:64, 128:256], in1=wb2[:])
            nc.sync.dma_start(out=wd2[64:65, :], in_=b_m1[:].unsqueeze(0))

            w2bf = P.tile([HID, HID], BF16, tag="w2bf")
            w2f = P.tile([HID, HID], F32, tag="w2f")
            nc.sync.dma_start(out=w2f[:], in_=W_m2[:])
            nc.vector.tensor_copy(out=w2bf[:], in_=w2f[:])

            # identity (bf16) for the v pass-through matmul: iota col-index
            # vs per-partition row-index, compared on DVE
            icol = P.tile([128, 128], I16, tag="icol")
            nc.gpsimd.iota(icol[:], pattern=[[1, 128]], base=0,
                           channel_multiplier=0)
            irow = P.tile([128, 1], I16, tag="irow")
            nc.gpsimd.iota(irow[:], pattern=[[0, 1]], base=0,
                           channel_multiplier=1)
            i_bf = P.tile([128, 128], BF16, tag="i_bf")
            nc.vector.tensor_tensor(out=i_bf[:], in0=icol[:],
                                    in1=irow[:, 0:1].to_broadcast([128, 128]),
                                    op=OP.is_equal)

            # R one-hot repeat matrix [GC, GC*K]: R[n, e] = (e//K == n)
            rcol = P.tile([GC, GC * K], I16, tag="rcol")
            nc.gpsimd.iota(rcol[:].rearrange("p (c k) -> p c k", k=K),
                           pattern=[[1, GC], [0, K]], base=0,
                           channel_multiplier=0)
            r_bf = P.tile([GC, GC * K], BF16, tag="r_bf")
            nc.vector.tensor_tensor(out=r_bf[:], in0=rcol[:],
                                    in1=irow[0:GC, 0:1].to_broadcast([GC, GC * K]),
                                    op=OP.is_equal)

            # ---------------- inputs ----------------
            xsm = P.tile([BSL, NB], F32, tag="xsm")
            ysm = P.tile([BSL, NB], F32, tag="ysm")
            st3 = state[:].rearrange("s (i c) -> s i c", c=2)
            nc.sync.dma_start(out=xsm[:], in_=st3[:, :, 0])
            nc.sync.dma_start(out=ysm[:], in_=st3[:, :, 1])

            inp4 = P.tile([4, NL], F32, tag="inp4")
            nc.sync.dma_start(
                out=inp4[2:4, :].rearrange("c (s i) -> c s i", s=BSL),
                in_=state[:].rearrange("s (i c) -> c s i", c=2),
            )
            nc.sync.dma_start(out=inp4[0:2, :], in_=tar[:].rearrange("n c -> c n"))
            nc.scalar.activation(out=inp4[0:2, :], in_=inp4[0:2, :], func=AF.Tanh)

            # ---------------- class features (per-sample constant) --------
            embt = P.tile([EMB, 3], F32, tag="embt")
            nc.sync.dma_start(out=embt[:], in_=emb_tab[:].rearrange("c h -> h c"))
            nc.scalar.activation(out=embt[:], in_=embt[:], func=AF.Tanh)
            embr = P.tile([EMB, NB], F32, tag="embr")
            for c in range(3):
                nc.vector.tensor_copy(
                    out=embr[:, c * 10:(c + 1) * 10],
                    in_=embt[:, c:c + 1].to_broadcast([EMB, 10]),
                )
            feat_b = P.tile([65, NL], F32, tag="feat_b")
            nc.vector.memset(feat_b[64:65, :], 1.0)  # ones row (bias trick)

            # ---------------- dense MLP to feat ----------------
            h1 = P.tile([HID, NL], F32, tag="h1")
            feat_a = P.tile([HID, NL], F32, tag="feat_a")
            with tc.tile_pool(name="psA", bufs=2, space="PSUM") as PSA:
                cps = PSA.tile([EMB, NB], F32, tag="cps")
                nc.tensor.matmul(out=cps[:], lhsT=w_emb[:], rhs=embr[:],
                                 start=True, stop=True)
                cls30 = P.tile([EMB, NB], F32, tag="cls30")
                nc.scalar.activation(out=cls30[:], in_=cps[:], func=AF.Tanh,
                                     bias=bem[:, 0:1])
                # broadcast 30-col class block to all samples
                nc.vector.tensor_copy(
                    out=feat_b[0:64, :].rearrange("p (s i) -> p s i", s=BSL),
                    in_=cls30[:].unsqueeze(1).broadcast_to([EMB, BSL, NB]),
                )
                for n0 in range(0, NL, NT):
                    ps = PSA.tile([HID, NT], F32, tag="ps")
                    nc.tensor.matmul(out=ps[:], lhsT=w_in1[:],
                                     rhs=inp4[:, n0:n0 + NT], start=True, stop=True)
                    nc.scalar.activation(out=h1[:, n0:n0 + NT], in_=ps[:],
                                         func=AF.Tanh, bias=bi1[:, 0:1])
                for n0 in range(0, NL, NT):
                    ps = PSA.tile([HID, NT], F32, tag="ps")
                    nc.tensor.matmul(out=ps[:], lhsT=w_in2[:],
                                     rhs=h1[:, n0:n0 + NT], start=True, stop=True)
                    nc.scalar.activation(out=feat_a[:, n0:n0 + NT], in_=ps[:],
                                         func=AF.Tanh, bias=bi2[:, 0:1])

                # ---------------- u (64-node tiles, bf16, SBUF) & v table --
                # u_sb[r, g*128+h] = u[64g + r, h]  (so every edge group's
                # lhsT slice starts at partition 0)
                u_sb = P.tile([64, NG * 128], BF16, tag="u_sb")
                for t in range(NG // 4):
                    pu = PSA.tile([64, 512], F32, tag="pu")
                    for s in range(4):
                        n0 = (4 * t + s) * 64
                        nc.tensor.matmul(out=pu[:, s * 128:(s + 1) * 128],
                                         lhsT=feat_a[:, n0:n0 + 64],
                                         rhs=wd1[:], start=True, stop=False)
                        nc.tensor.matmul(out=pu[:, s * 128:(s + 1) * 128],
                                         lhsT=feat_b[:, n0:n0 + 64],
                                         rhs=wd2[:], start=False, stop=True)
                    nc.vector.tensor_copy(out=u_sb[:, t * 512:(t + 1) * 512],
                                          in_=pu[:])
                v_st = P.tile([128, NL], BF16, tag="v_st")
                for t in range(NL // 128):
                    n0 = t * 128
                    pv = PSA.tile([128, 128], F32, tag="pv")
                    nc.tensor.matmul(out=pv[:], lhsT=feat_a[:, n0:n0 + 128],
                                     rhs=wb1[:], start=True, stop=False)
                    nc.tensor.matmul(out=pv[:], lhsT=feat_b[0:64, n0:n0 + 128],
                                     rhs=wb2[:], start=False, stop=True)
                    nc.vector.tensor_copy(out=v_st[:, n0:n0 + 128], in_=pv[:])
            nc.sync.dma_start(
                out=v_dram[:].rearrange("(t p) h -> p t h", p=128),
                in_=v_st[:].rearrange("p (t h) -> p t h", h=128),
            )

            # ---------------- knn ----------------
            dsm = P.tile([BSL, NB * NB], F32, tag="dsm")
            d3 = dsm[:].rearrange("s (i j) -> s i j", i=NB)
            xi = xsm[:].unsqueeze(2).broadcast_to([BSL, NB, NB])
            xj = xsm[:].unsqueeze(1).broadcast_to([BSL, NB, NB])
            yi = ysm[:].unsqueeze(2).broadcast_to([BSL, NB, NB])
            yj = ysm[:].unsqueeze(1).broadcast_to([BSL, NB, NB])
            dx = P.tile([BSL, NB * NB], F32, tag="dx")
            dx3 = dx[:].rearrange("s (i j) -> s i j", i=NB)
            nc.vector.tensor_sub(out=dx3, in0=xi, in1=xj)
            nc.vector.tensor_mul(out=dx3, in0=dx3, in1=dx3)
            nc.vector.tensor_sub(out=d3, in0=yi, in1=yj)
            nc.vector.tensor_mul(out=d3, in0=d3, in1=d3)
            nc.vector.tensor_add(out=d3, in0=d3, in1=dx3)
            # negate so vector.max finds the k smallest; self-distance 0 is
            # then always the single largest -> dropped as slot 0 of round 1.
            nc.vector.tensor_scalar_mul(dsm[:], dsm[:], -1.0)

            # relayout to center-major [4-sample groups]: dcm[(q,i),(t,j)]
            dcm = P.tile([120, 32 * NB], F32, tag="dx")
            dsm4 = dsm[:].rearrange("(t q) (i j) -> t q i j", q=4, j=NB)
            dcm3 = dcm[:].rearrange("p (t j) -> p t j", j=NB)
            for t in range(32):
                nc.sync.dma_start(out=dcm3[:, t, :], in_=dsm4[t])

            idxg = P.tile([120, 32 * K], U16, tag="idxg")
            pbrow = P.tile([1, 4], U16, tag="pbrow")
            for q in range(4):
                nc.vector.memset(pbrow[:, q:q + 1], q * 30)
            pbase = P.tile([120, 1], U16, tag="pbase")
            pb4 = pbase[:].rearrange("(q r) c -> q r c", r=30)
            for q in range(4):
                nc.sync.dma_start(
                    out=pb4[q],
                    in_=pbrow[:, q:q + 1].to_broadcast([1, 30]),
                )
            with tc.tile_pool(name="knn", bufs=3) as KP:
                for t in range(32):
                    dn = dcm[:, t * NB:(t + 1) * NB]
                    sc = KP.tile([120, 16], U16, tag="sc")
                    m8 = KP.tile([120, 8], F32, tag="m8")
                    nc.vector.max(out=m8[:], in_=dn)
                    nc.vector.max_index(out=sc[:, 0:8], in_max=m8[:], in_values=dn)
                    rep = KP.tile([120, NB], F32, tag="rep")
                    nc.vector.match_replace(out=rep[:], in_to_replace=m8[:],
                                            in_values=dn, imm_value=-3e38)
                    m2 = KP.tile([120, 8], F32, tag="m2")
                    nc.vector.max(out=m2[:], in_=rep[:])
                    nc.vector.max_index(out=sc[:, 8:16], in_max=m2[:], in_values=rep[:])
                    nc.vector.tensor_copy(out=idxg[:, t * K:(t + 1) * K],
                                          in_=sc[:, 1:11])
            toff = P.tile([120, 32 * K], U16, tag="toff")
            nc.gpsimd.iota(toff[:].rearrange("p (t k) -> p t k", t=32),
                           pattern=[[120, 32], [0, K]], base=0, channel_multiplier=0)
            nc.vector.tensor_add(out=idxg[:], in0=idxg[:],
                                 in1=pbase[:, 0:1].to_broadcast([120, 32 * K]))
            nc.vector.tensor_add(out=idxg[:], in0=idxg[:], in1=toff[:])
            nc.sync.dma_start(
                out=idx_dram[:].rearrange("(t p k) -> p t k", p=120, k=K),
                in_=idxg[:].bitcast(I16).rearrange("p (t k) -> p t k", k=K),
            )
            # reload in dma_gather index layout: [128, NE//16] int16,
            # idx e at (e%16, e//16), replicated to all 8 16-partition blocks
            idx16 = P.tile([128, NE // 16], I16, tag="idx16")
            for r in range(8):
                nc.sync.dma_start(
                    out=idx16[16 * r:16 * (r + 1), :],
                    in_=idx_dram[:].rearrange("(c p) -> p c", p=16),
                )

            # ---------------- edge pipeline ----------------
            xpre = P.tile([HID, NL], F32, tag="xpre")
            with (
                tc.tile_pool(name="zp", bufs=2, space="PSUM") as ZP,
                tc.tile_pool(name="mp", bufs=2, space="PSUM") as MP,
                tc.tile_pool(name="ep", bufs=3) as EP,
            ):
                for g in range(NG):
                    e0 = g * GC * K          # 640 edges per group
                    zv = EP.tile([128, GC * K], BF16, tag="zv")
                    nc.gpsimd.dma_gather(
                        out_ap=zv[:].unsqueeze(1),
                        in_ap=v_dram[:],
                        idxs_ap=idx16[:, e0 // 16:(e0 + GC * K) // 16],
                        num_idxs=GC * K, num_idxs_reg=nreg,
                        elem_size=HID, transpose=True,
                    )
                    zp = ZP.tile([HID, GC * K], F32, tag="zp")
                    ul = u_sb[:, g * 128:(g + 1) * 128]
                    for c0 in range(0, GC * K, 512):
                        c1 = min(c0 + 512, GC * K)
                        nc.tensor.matmul(out=zp[:, c0:c1], lhsT=ul,
                                         rhs=r_bf[:, c0:c1], start=True, stop=False)
                        nc.tensor.matmul(out=zp[:, c0:c1], lhsT=i_bf[:],
                                         rhs=zv[:, c0:c1], start=False, stop=True)
                    tf = EP.tile([HID, GC * K], BF16, tag="tf")
                    nc.scalar.activation(out=tf[:], in_=zp[:], func=AF.Tanh)
                    mp = MP.tile([HID, GC * K], F32, tag="mp")
                    for c0 in range(0, GC * K, 512):
                        c1 = min(c0 + 512, GC * K)
                        nc.tensor.matmul(out=mp[:, c0:c1], lhsT=w2bf[:],
                                         rhs=tf[:, c0:c1], start=True, stop=True)
                    nc.vector.reduce_max(
                        out=xpre[:, g * GC:(g + 1) * GC],
                        in_=mp[:].rearrange("p (c k) -> p c k", k=K),
                        axis=mybir.AxisListType.X,
                    )

            # ---------------- actor head ----------------
            xf = xpre
            nc.scalar.activation(out=xf[:], in_=xpre[:], func=AF.Tanh,
                                 bias=bm2[:, 0:1])
            a1 = P.tile([HID, NL], F32, tag="h1")
            mustd = P.tile([2, 2 * NL], F32, tag="mustd")
            nls = P.tile([2, 1], F32, tag="nls")
            nc.vector.memset(nls[:], -1.5)
            with tc.tile_pool(name="psH", bufs=2, space="PSUM") as PSH:
                for n0 in range(0, NL, NT):
                    ps = PSH.tile([HID, NT], F32, tag="psh")
                    nc.tensor.matmul(out=ps[:], lhsT=w_a1[:],
                                     rhs=xf[:, n0:n0 + NT], start=True, stop=True)
                    nc.scalar.activation(out=a1[:, n0:n0 + NT], in_=ps[:],
                                         func=AF.Tanh, bias=ba1[:, 0:1])
                for n0 in range(0, NL, NT):
                    pmu = PSH.tile([2, NT], F32, tag="pmu")
                    nc.tensor.matmul(out=pmu[:], lhsT=w_a2[:, 0:2],
                                     rhs=a1[:, n0:n0 + NT], start=True, stop=True)
                    nc.scalar.activation(out=mustd[:, n0:n0 + NT], in_=pmu[:],
                                         func=AF.Tanh, bias=ba2[:, 0:1])
                    # std = exp(3.5*tanh(ls) - 1.5)
                    pls = PSH.tile([2, NT], F32, tag="pls")
                    nc.tensor.matmul(out=pls[:], lhsT=w_a2[:, 2:4],
                                     rhs=a1[:, n0:n0 + NT], start=True, stop=True)
                    ls = PSH.tile([2, NT], F32, tag="ls")
                    nc.scalar.activation(out=ls[:], in_=pls[:],
                                         func=AF.Tanh, bias=ba2ls[:, 0:1])
                    nc.scalar.activation(out=mustd[:, NL + n0:NL + n0 + NT],
                                         in_=ls[:], func=AF.Exp,
                                         bias=nls[:, 0:1], scale=3.5)
            nc.vector.tensor_add(out=mustd[:, 0:NL], in0=mustd[:, 0:NL],
                                 in1=inp4[0:2, :])
            nc.vector.tensor_scalar_mul(mustd[:, 0:NL], mustd[:, 0:NL], 0.3)

            # ---------------- output relayout + store ----------------
            ob = P.tile([BSL, 120], F32, tag="ob")  # [s, (o,i,c)] o in {mu,std}
            ob4 = ob[:].rearrange("s (o i c) -> o c s i", o=2, c=2)
            for o in range(2):
                for c in range(2):
                    nc.sync.dma_start(
                        out=ob4[o, c],
                        in_=mustd[c:c + 1, o * NL:(o + 1) * NL]
                        .rearrange("p (s i) -> p s i", s=BSL),
                    )
            nc.sync.dma_start(
                out=out[:].rearrange("o s d -> s o d"),
                in_=ob[:].rearrange("s (o d) -> s o d", o=2),
            )
    if not nc.is_finalized():
        nc.finalize()
    return nc


_NC_CACHE = None


def get_nc():
    global _NC_CACHE
    if _NC_CACHE is None:
        _NC_CACHE = build_nc()
    return _NC_CACHE


def make_in_maps(inputs):
    reps = {k: np.asarray(v, dtype=np.float32) for k, v in inputs.items()}
    state = reps.pop("state_inp")
    tar = reps.pop("tar_scores")
    in_maps = []
    for i in range(NCORES):
        m = {k: v for k, v in reps.items()}
        m["state_inp"] = np.ascontiguousarray(state[i * BSL:(i + 1) * BSL])
        m["tar_scores"] = np.ascontiguousarray(tar[i * NL:(i + 1) * NL])
        in_maps.append(m)
    return in_maps


def kernel(**inputs) -> np.ndarray:
    from concourse.bass_utils import run_bass_kernel_spmd

    nc = get_nc()
    in_maps = make_in_maps(inputs)
    res = run_bass_kernel_spmd(nc, in_maps, core_ids=list(range(NCORES)))
    outs = res.results
    full = np.empty((2, BS, NB * 2), dtype=np.float32)
    for i in range(NCORES):
        o = outs[i]["out"] if isinstance(outs[i], dict) else outs[i][0]
        full[:, i * BSL:(i + 1) * BSL, :] = np.asarray(o).reshape(2, BSL, NB * 2)
    return full
